# revision 18
# baseline (speedup 1.0000x reference)
"""Trainium2 Bass kernel for CrossMultiheadAttention.

B=4, T=S=1024, E=1024, H=16, D=64. 8 NeuronCores.

Sharding: core c handles (batch b=c//2, T-half th=c%2) -> 512 query rows.
Each core computes k/v projections for its whole batch (duplicated between
the 2 cores sharing a batch), all 16 heads of attention for its queries and
the full output projection for its rows. Output gather is a pure concat.

v2 design notes (vs the first working version):
  * The tensor engine only reaches its 2.4 GHz p-state after ~3us of
    back-to-back work; any gap drops it to 1.2 GHz. So the whole kernel is
    emitted as one continuous PE stream: K-projection chunks are interleaved
    INTO the V-projection and attention groups, score matmuls run two
    head-pair groups ahead of the attn@v matmuls, and the exp/mult chain
    runs on ACT/DVE far off the PE critical path.
  * exp(attn_bias) * (mask ? 0 : 1) is precomputed on the HOST in bf16.
    The device computes e = exp(scores) [ACT, N=1024 across a 2-bank PSUM
    pair] then e *= eb in place [DVE, bf16x2 mode] - no fp32 tensor_tensor
    ADD (1x mode) and no mask handling on the device.
  * All PSUM work uses [128, 1024] two-bank pair tiles so activations
    amortize ACT's 352-cycle fixed overhead. PSUM budget: flowing pairs
    (2x2 banks) + held K-projection pair (2) + attn@v accumulators (2) = 8.
  * Softmax denominators come from the v65 ones-column; reciprocals run on
    DVE over a [64, 64] DRAM-reshaped view (64 elems/lane, not 512/lane).
"""
import sys

sys.path.insert(0, "/opt/trn_rl_repo")

import numpy as np
import ml_dtypes

import concourse.bass as bass
import concourse.bacc as bacc
import concourse.tile as tile
from concourse import mybir
from concourse.bass_utils import run_bass_kernel_spmd


def _pbcast(ap, nparts):
    """View `ap` (a [1, N] row) replicated across nparts partitions via a
    0-stride partition dim — DMA-source only."""
    row = ap
    return bass.AP(tensor=row.tensor, offset=row.offset,
                   ap=[[0, nparts]] + [list(d) for d in row.ap[1:]])


def _dview(ap, p, f):
    """Flat [p, f] view of a contiguous DRAM region starting at `ap`."""
    return bass.AP(tensor=ap.tensor, offset=ap.offset, ap=[[f, p], [1, f]])

F32 = mybir.dt.float32
BF16 = mybir.dt.bfloat16
Act = mybir.ActivationFunctionType
Alu = mybir.AluOpType
NPBF16 = ml_dtypes.bfloat16

B, T, S, E, H, D = 4, 1024, 1024, 1024, 16, 64
HP = H // 2          # head pairs
TS = T // 2          # per-core query rows (t-shard)
ET = E // 128        # 128-row tiles of the embed dim
SCALING = D ** -0.5

_CACHE = {}


def build_nc():
    nc = bacc.Bacc("TRN2", target_bir_lowering=False, debug=False, num_devices=8)

    qin_d = nc.dram_tensor("qin", [E, TS], BF16, kind="ExternalInput").ap()
    kin_d = nc.dram_tensor("kin", [E, S], BF16, kind="ExternalInput").ap()
    vin_d = nc.dram_tensor("vin", [E, S], BF16, kind="ExternalInput").ap()
    # exp(bias)*mask, [hp*8+j, s-in-tile, hh*512 + t]
    eb_d = nc.dram_tensor("ebT", [HP * 8, 128, 2 * TS], BF16,
                          kind="ExternalInput").ap()
    wq_d = nc.dram_tensor("wqt", [E, E], BF16, kind="ExternalInput").ap()
    wk_d = nc.dram_tensor("wkt", [E, E], BF16, kind="ExternalInput").ap()
    wv_d = nc.dram_tensor("wvt", [E, E], BF16, kind="ExternalInput").ap()
    wo_d = nc.dram_tensor("wot", [E, E], BF16, kind="ExternalInput").ap()
    bq_d = nc.dram_tensor("bqs", [128, 8], F32, kind="ExternalInput").ap()
    bk_d = nc.dram_tensor("bks", [128, 8], F32, kind="ExternalInput").ap()
    bv_d = nc.dram_tensor("bvr", [1, E], BF16, kind="ExternalInput").ap()
    bo_d = nc.dram_tensor("bor", [1, E], BF16, kind="ExternalInput").ap()
    out_d = nc.dram_tensor("out", [TS, E], F32, kind="ExternalOutput").ap()

    with tile.TileContext(nc) as tc:
        with tc.tile_pool(name="consts", bufs=1) as consts, \
             tc.tile_pool(name="wring", bufs=8) as wring, \
             tc.tile_pool(name="wkp", bufs=1) as wkp, \
             tc.tile_pool(name="kinp", bufs=1) as kinp, \
             tc.tile_pool(name="qvin", bufs=16) as qvin, \
             tc.tile_pool(name="persist", bufs=1) as persist, \
             tc.tile_pool(name="ebp", bufs=8) as ebp, \
             tc.tile_pool(name="emp", bufs=18) as emp, \
             tc.tile_pool(name="small", bufs=2) as small, \
             tc.tile_pool(name="osbp", bufs=2) as osbp, \
             tc.tile_pool(name="dramp", bufs=1, space="DRAM") as dramp, \
             tc.tile_pool(name="pssc", bufs=2, space="PSUM") as pssc, \
             tc.tile_pool(name="psot", bufs=1, space="PSUM") as psot:

            # ---- constants (bvb/bob DMAs deferred; see stages B/C) ----
            bq_sb = consts.tile([128, 8], F32, tag="bq")
            bk_sb = consts.tile([128, 8], F32, tag="bk")
            bvb = consts.tile([128, E], BF16, tag="bvb")  # bv bcast over parts
            bob = consts.tile([128, E], BF16, tag="bob")  # bo bcast over parts

            # ---- persistent products ----
            qT = [persist.tile([128, TS], BF16, tag=f"qt{hp}", name=f"qt{hp}")
                  for hp in range(HP)]
            kT = [[persist.tile([128, 512], BF16, tag=f"kt{hp}_{sh}",
                                name=f"kt{hp}_{sh}")
                   for sh in range(2)] for hp in range(HP)]
            v65 = [persist.tile([128, H, 65], BF16, tag=f"v65_{j}",
                                name=f"v65_{j}")
                   for j in range(8)]
            # unnormalized oT (f32) and normalized bf16 copy for out-proj
            otn = [persist.tile([128, TS], F32, tag=f"otn{et}", name=f"otn{et}")
                   for et in range(ET)]
            otn2 = [persist.tile([128, TS], BF16, tag=f"otn2_{et}",
                                 name=f"otn2_{et}")
                    for et in range(ET)]
            den_dram = dramp.tile([16, TS], F32, tag="dend", name="dend")
            rcp_dram = dramp.tile([16, TS], F32, tag="rcpd", name="rcpd")

            # ---- weight / input loads (stage A prefetch) ----
            # First-needed tiles are split into column chunks across several
            # DMA queues so the first matmul can start ~2us after queue
            # start-up instead of waiting one queue to stream 256KB.
            def chunked_load(t_, src, n):
                w = src.ap[-1][1]
                for ch in range(n):
                    lo, hi = w * ch // n, w * (ch + 1) // n
                    nc.sync.dma_start(out=t_[:, lo:hi], in_=src[:, lo:hi])

            wq_sb = [wring.tile([128, E], BF16, tag="w", name="w")
                     for _ in range(ET)]
            qin_sb = [qvin.tile([128, TS], BF16, tag="qv", name="qv")
                      for _ in range(ET)]
            wk_sb = [wkp.tile([128, E], BF16, tag=f"wk{et}", name=f"wk{et}")
                     for et in range(ET)]
            kin_sb = [[kinp.tile([128, 512], BF16, tag=f"ki{sh}_{et}",
                                 name=f"ki{sh}_{et}") for et in range(ET)]
                      for sh in range(2)]
            chunked_load(qin_sb[0], qin_d[0:128, :], 4)
            chunked_load(wq_sb[0], wq_d[0:128, :], 4)
            chunked_load(qin_sb[1], qin_d[128:256, :], 2)
            chunked_load(wq_sb[1], wq_d[128:256, :], 2)
            nc.sync.dma_start(out=bq_sb, in_=bq_d)
            nc.sync.dma_start(out=bk_sb, in_=bk_d)
            for et in range(2, ET):
                nc.sync.dma_start(out=qin_sb[et],
                                  in_=qin_d[et * 128:(et + 1) * 128, :])
                nc.sync.dma_start(out=wq_sb[et],
                                  in_=wq_d[et * 128:(et + 1) * 128, :])
            for et in range(ET):
                nc.sync.dma_start(out=wk_sb[et],
                                  in_=wk_d[et * 128:(et + 1) * 128, :])
                nc.sync.dma_start(out=kin_sb[0][et],
                                  in_=kin_d[et * 128:(et + 1) * 128, 0:512])
            for et in range(ET):
                nc.sync.dma_start(out=kin_sb[1][et],
                                  in_=kin_d[et * 128:(et + 1) * 128, 512:1024])

            # v65 ones columns (softmax denominator trick)
            for j in range(8):
                nc.vector.memset(v65[j][:, :, 64:65], 1.0)

            def k_proj_mms(hp, ps, sh, et0, n=2):
                """n et-steps of K[hp]'s sh-half into pair half sh."""
                for et in range(et0, et0 + n):
                    nc.tensor.matmul(ps[:, sh * 512:(sh + 1) * 512],
                                     wk_sb[et][:, hp * 128:(hp + 1) * 128],
                                     kin_sb[sh][et], start=(et == 0),
                                     stop=(et == ET - 1))

            def k_evac(hp, ps, engine):
                for sh in range(2):
                    if engine == "act":
                        nc.scalar.activation(kT[hp][sh],
                                             ps[:, sh * 512:(sh + 1) * 512],
                                             Act.Identity,
                                             bias=bk_sb[:, hp:hp + 1])
                    else:
                        nc.vector.tensor_scalar(
                            out=kT[hp][sh], in0=ps[:, sh * 512:(sh + 1) * 512],
                            scalar1=bk_sb[:, hp:hp + 1], scalar2=None,
                            op0=Alu.add)

            # ---- stage A: Q projection with K[0..1] woven in ----
            def q_pair(hpp):
                ps = pssc.tile([128, 1024], F32, tag="pair", name="pair")
                for h2 in range(2):
                    hp = 2 * hpp + h2
                    for et in range(ET):
                        nc.tensor.matmul(ps[:, h2 * 512:(h2 + 1) * 512],
                                         wq_sb[et][:, hp * 128:(hp + 1) * 128],
                                         qin_sb[et], start=(et == 0),
                                         stop=(et == ET - 1))
                for h2 in range(2):
                    hp = 2 * hpp + h2
                    nc.scalar.activation(qT[hp], ps[:, h2 * 512:(h2 + 1) * 512],
                                         Act.Identity, bias=bq_sb[:, hp:hp + 1])

            q_pair(0)
            kps = psot.tile([128, 1024], F32, tag="kpair", name="kpair")
            for sh in range(2):
                k_proj_mms(0, kps, sh, 0, n=ET)
            k_evac(0, kps, "act")
            q_pair(1)
            q_pair(2)
            kps = psot.tile([128, 1024], F32, tag="kpair", name="kpair")
            for sh in range(2):
                k_proj_mms(1, kps, sh, 0, n=ET)
            k_evac(1, kps, "act")
            q_pair(3)

            # ---- attention building blocks ----
            e2m_tiles = {}

            def s_pair(hp, j):
                """Scores for both heads of hp at s-tile j -> exp -> *=eb."""
                sh, sl = j // 4, j % 4
                eb = ebp.tile([128, 1024], BF16, tag="eb", name="eb")
                nc.sync.dma_start(out=eb, in_=eb_d[hp * 8 + j])
                ps = pssc.tile([128, 1024], F32, tag="pair", name="pair")
                for hh in range(2):
                    nc.tensor.matmul(
                        ps[:, hh * 512:(hh + 1) * 512],
                        kT[hp][sh][hh * 64:(hh + 1) * 64,
                                   sl * 128:(sl + 1) * 128],
                        qT[hp][hh * 64:(hh + 1) * 64, :],
                        start=True, stop=True,
                        tile_position=(hh * 64, 0))
                em = emp.tile([128, 1024], BF16, tag="em", name="em")
                nc.scalar.activation(em, ps, Act.Exp)
                nc.vector.tensor_tensor(out=em, in0=em, in1=eb, op=Alu.mult)
                e2m_tiles[(hp, j)] = em

            def o_mm(hp, j, poT):
                em = e2m_tiles.pop((hp, j))
                for hh in range(2):
                    h = 2 * hp + hh
                    nc.tensor.matmul(poT[hh], v65[j][:, h, :],
                                     em[:, hh * 512:(hh + 1) * 512],
                                     start=(j == 0), stop=(j == 7))

            def po_evac(hp, poT):
                for hh in range(2):
                    h = 2 * hp + hh
                    nc.vector.tensor_copy(out=otn[hp][hh * 64:(hh + 1) * 64, :],
                                          in_=poT[hh][0:64, :])
                    dst = small.tile([65, TS], F32, tag="dst", name="dst")
                    nc.vector.tensor_copy(out=dst[64:65, :],
                                          in_=poT[hh][64:65, :])
                    nc.gpsimd.dma_start(out=den_dram[h:h + 1, :],
                                        in_=dst[64:65, :])

            def norm_mini(hp):
                """Reciprocal + broadcast + normalize for head-pair hp,
                emitted right after po_evac(hp) so otn2[hp] is ready ~3us
                after the group ends. All DMAs ride the gpsimd (software
                DGE) path so they never contend with the eb/weight stream."""
                deng = small.tile([16, 64], F32, tag="deng", name="deng")
                nc.gpsimd.dma_start(out=deng,
                                    in_=_dview(den_dram[2 * hp:2 * hp + 2, :],
                                               16, 64))
                rcpg = small.tile([16, 64], F32, tag="rcpg", name="rcpg")
                nc.vector.reciprocal(out=rcpg, in_=deng)
                nc.gpsimd.dma_start(out=_dview(rcp_dram[2 * hp:2 * hp + 2, :],
                                               16, 64),
                                    in_=rcpg)
                bc = small.tile([128, TS], F32, tag="bc", name="bc")
                row = rcp_dram[2 * hp:2 * hp + 2, :]
                nc.gpsimd.dma_start(
                    out=bc,
                    in_=bass.AP(tensor=row.tensor, offset=row.offset,
                                ap=[[TS, 2], [0, 64], [1, TS]]))
                nc.vector.tensor_tensor(out=otn2[hp], in0=otn[hp], in1=bc,
                                        op=Alu.mult)

            # ---- stage B: V projection + K[2..3] + scores for hp 0..1 ----
            chunked_load(bvb, _pbcast(bv_d, 128), 2)
            wv_sb = []
            for et in range(ET):
                w = wring.tile([128, E], BF16, tag="w", name="w")
                nc.sync.dma_start(out=w, in_=wv_d[et * 128:(et + 1) * 128, :])
                wv_sb.append(w)
            vin_sb = [[None] * ET for _ in range(2)]
            for sh in range(2):
                for et in range(ET):
                    t_ = qvin.tile([128, 512], BF16, tag="qv", name="qv")
                    nc.sync.dma_start(
                        out=t_, in_=vin_d[et * 128:(et + 1) * 128,
                                          sh * 512:(sh + 1) * 512])
                    vin_sb[sh][et] = t_

            spairsB = [(0, j) for j in range(8)] + [(1, j) for j in range(8)]
            kps = psot.tile([128, 1024], F32, tag="kpair", name="kpair")
            vpair = None
            for idx in range(16):
                j, half = idx // 2, idx % 2
                sh, st = j // 4, j % 4
                if half == 0:
                    vpair = pssc.tile([128, 1024], F32, tag="pair", name="pair")
                for et in range(ET // 2 * half, ET // 2 * (half + 1)):
                    for ih in range(2):
                        nc.tensor.matmul(
                            vpair[:, ih * 512:(ih + 1) * 512],
                            vin_sb[sh][et][:, st * 128:(st + 1) * 128],
                            wv_sb[et][:, ih * 512:(ih + 1) * 512],
                            start=(et == 0), stop=(et == ET - 1))
                if half == 1:
                    nc.vector.tensor_tensor(
                        out=v65[j][:, :, 0:64],
                        in0=vpair.rearrange("p (h d) -> p h d", h=16),
                        in1=bvb.rearrange("p (h d) -> p h d", h=16),
                        op=Alu.add)
                # K[2] over idx 0..7, K[3] over idx 8..15 (2 mms per idx)
                if idx == 8:
                    k_evac(2, kps, "dve")
                    kps = psot.tile([128, 1024], F32, tag="kpair",
                                    name="kpair")
                khp = 2 + idx // 8
                k_proj_mms(khp, kps, (idx % 8) // 4, 2 * (idx % 4), n=2)
                s_pair(*spairsB[idx])
            k_evac(3, kps, "dve")

            # ---- stage C: attention groups; K[4..7] and out-proj wave A
            # (et 0..3 of tt=g-4, using PSUM freed by the K projections)
            # interleave into the groups ----
            chunked_load(bob, _pbcast(bo_d, 128), 2)
            wo_sb = []
            for et in range(ET):
                w = wring.tile([128, E], BF16, tag="w", name="w")
                nc.sync.dma_start(out=w, in_=wo_d[et * 128:(et + 1) * 128, :])
                wo_sb.append(w)

            def out_mm1(tt, ps, et, start_et, stop_et):
                for ih in range(2):
                    nc.tensor.matmul(
                        ps[:, ih * 512:(ih + 1) * 512],
                        otn2[et][:, tt * 128:(tt + 1) * 128],
                        wo_sb[et][:, ih * 512:(ih + 1) * 512],
                        start=(et == start_et), stop=(et == stop_et))

            def out_store(tt, ps, part=None):
                for ih in range(2):
                    o = osbp.tile([128, 512], F32, tag="osb", name="osb")
                    nc.vector.tensor_tensor(
                        out=o, in0=ps[:, ih * 512:(ih + 1) * 512],
                        in1=(bob if part is None else part)[
                            :, ih * 512:(ih + 1) * 512], op=Alu.add)
                    nc.sync.dma_start(
                        out=out_d[tt * 128:(tt + 1) * 128,
                                  ih * 512:(ih + 1) * 512], in_=o)

            wave_ps = {}
            parts = {}
            for g in range(8):
                poT = [psot.tile([65, 512], F32, tag=f"ot{hh}", name=f"ot{hh}")
                       for hh in range(2)]
                kpair = None
                if g + 4 <= 7:
                    kpair = psot.tile([128, 1024], F32, tag="kpair",
                                      name="kpair")
                for j in range(8):
                    # last slot: o first so po_evac leads the group's DVE tail
                    if j == 7:
                        if kpair is not None:
                            k_proj_mms(g + 4, kpair, j // 4, 2 * (j % 4))
                        o_mm(g, j, poT)
                        po_evac(g, poT)
                        norm_mini(g)
                        if g + 2 <= 7:
                            s_pair(g + 2, j)
                    else:
                        if g + 2 <= 7:
                            s_pair(g + 2, j)
                        if kpair is not None:
                            k_proj_mms(g + 4, kpair, j // 4, 2 * (j % 4))
                        o_mm(g, j, poT)
                    # out-proj wave A: tt=g-4 covers et 0..g-1 (otn2[et] is
                    # ready ~one group after et's own group), one et per slot.
                    # tt0/tt1 use the kpair banks (K projections done), tt2/
                    # tt3 the pair ring (score pairs done after group 5).
                    if g >= 4 and j < g:
                        tt = g - 4
                        if j == 0:
                            wave_ps[tt] = (
                                psot.tile([128, 1024], F32, tag="kpair",
                                          name="kpair") if tt in (0, 1)
                                else pssc.tile([128, 1024], F32, tag="pair",
                                               name="pair"))
                        # tt0/tt2 close their group (partial spilled);
                        # tt1/tt3 stay open and finish in stage D.
                        out_mm1(tt, wave_ps[tt], j, 0,
                                g - 1 if tt in (0, 2) else ET - 1)
                if kpair is not None:
                    k_evac(g + 4, kpair, "dve")
                # tt0 would block tt1's wave alloc on the kpair ring (and tt2
                # tt3's on the pair ring): spill tt0/tt2 partials to SBUF;
                # tt1/tt3 pairs stay held into stage D.
                if g in (4, 6):
                    tt = g - 4
                    part = osbp.tile([128, 1024], BF16, tag="part",
                                     name="part")
                    nc.vector.tensor_tensor(out=part, in0=wave_ps[tt],
                                            in1=bob, op=Alu.add)
                    parts[tt] = part

            # ---- stage D: out-proj tails (tt0: et4.., tt1: et5.., tt3: et7,
            # then tt2: et6.. on tt3's freed pair-ring slot) ----
            tail_ps = {1: wave_ps[1], 3: wave_ps[3]}
            tail_ps[0] = pssc.tile([128, 1024], F32, tag="pair", name="pair")
            for et in range(4, 7):
                out_mm1(0, tail_ps[0], et, 4, ET - 1)
            for et in range(5, 7):
                out_mm1(1, tail_ps[1], et, 0, ET - 1)
            out_mm1(0, tail_ps[0], 7, 4, ET - 1)
            out_store(0, tail_ps[0], part=parts[0])
            out_mm1(1, tail_ps[1], 7, 0, ET - 1)
            out_store(1, tail_ps[1])
            out_mm1(3, tail_ps[3], 7, 0, ET - 1)
            out_store(3, tail_ps[3])
            tail_ps[2] = pssc.tile([128, 1024], F32, tag="pair", name="pair")
            for et in range(6, ET):
                out_mm1(2, tail_ps[2], et, 6, ET - 1)
            out_store(2, tail_ps[2], part=parts[2])

    nc.compile()
    return nc


def _prepare_in_maps(query, key, value, key_padding_mask, attn_bias,
                     wq, bq, wk, bk, wv, bv, wo, bo):
    wqt = (np.ascontiguousarray(wq.T) * SCALING).astype(NPBF16)
    wkt = np.ascontiguousarray(wk.T).astype(NPBF16)
    wvt = np.ascontiguousarray(wv.T).astype(NPBF16)
    wot = np.ascontiguousarray(wo.T).astype(NPBF16)
    bqs = np.ascontiguousarray((bq * SCALING).reshape(8, 128).T)
    bks = np.ascontiguousarray(bk.astype(np.float32).reshape(8, 128).T)
    bvr = np.ascontiguousarray(bv.astype(NPBF16))[None, :]
    bor = np.ascontiguousarray(bo.astype(NPBF16))[None, :]

    kin_b = [np.ascontiguousarray(key[b_].T).astype(NPBF16) for b_ in range(B)]
    vin_b = [np.ascontiguousarray(value[b_].T).astype(NPBF16) for b_ in range(B)]

    # exp(bias) * (mask ? 0 : 1), bf16, per-core layout
    ebf = np.exp(attn_bias.astype(np.float32)).reshape(B, H, T, S)
    ebf = ebf * (~key_padding_mask)[:, None, None, :].astype(np.float32)

    in_maps = []
    for c in range(8):
        b_, th = c // 2, c % 2
        qin = np.ascontiguousarray(
            query[b_, th * TS:(th + 1) * TS, :].T).astype(NPBF16)
        sl = ebf[b_, :, th * TS:(th + 1) * TS, :]      # [16, 512t, 1024s]
        x = sl.reshape(8, 2, TS, 8, 128)               # [hp, hh, t, j, s128]
        x = x.transpose(0, 3, 4, 1, 2)                 # [hp, j, s128, hh, t]
        ebT = np.ascontiguousarray(x).reshape(64, 128, 1024).astype(NPBF16)
        in_maps.append({
            "qin": qin, "kin": kin_b[b_], "vin": vin_b[b_],
            "ebT": ebT,
            "wqt": wqt, "wkt": wkt, "wvt": wvt, "wot": wot,
            "bqs": bqs, "bks": bks, "bvr": bvr, "bor": bor,
        })
    return in_maps


def kernel(query, key, value, key_padding_mask, attn_bias,
           wq, bq, wk, bk, wv, bv, wo, bo, _run_kwargs=None):
    query = np.asarray(query, dtype=np.float32)
    key = np.asarray(key, dtype=np.float32)
    value = np.asarray(value, dtype=np.float32)
    key_padding_mask = np.asarray(key_padding_mask)
    attn_bias = np.asarray(attn_bias, dtype=np.float32)
    wq, bq = np.asarray(wq, np.float32), np.asarray(bq, np.float32)
    wk, bk = np.asarray(wk, np.float32), np.asarray(bk, np.float32)
    wv, bv = np.asarray(wv, np.float32), np.asarray(bv, np.float32)
    wo, bo = np.asarray(wo, np.float32), np.asarray(bo, np.float32)

    if "nc" not in _CACHE:
        _CACHE["nc"] = build_nc()
    nc = _CACHE["nc"]

    in_maps = _prepare_in_maps(query, key, value, key_padding_mask, attn_bias,
                               wq, bq, wk, bk, wv, bv, wo, bo)
    res = run_bass_kernel_spmd(nc, in_maps, core_ids=list(range(8)),
                               **(_run_kwargs or {}))
    _CACHE["last_results"] = res

    out = np.empty((B, T, E), dtype=np.float32)
    for c in range(8):
        b_, th = c // 2, c % 2
        out[b_, th * TS:(th + 1) * TS, :] = res.results[c]["out"]
    return out


# revision 19
# speedup vs baseline: 1.2060x; 1.2060x over previous
"""Trainium2 Bass kernel for CrossMultiheadAttention.

B=4, T=S=1024, E=1024, H=16, D=64. 8 NeuronCores.

Sharding: core c handles (batch b=c//2, T-half th=c%2) -> 512 query rows.
Each core computes k/v projections for its whole batch (duplicated between
the 2 cores sharing a batch), all 16 heads of attention for its queries and
the full output projection for its rows. Output gather is a pure concat.

v2 design notes (vs the first working version):
  * The tensor engine only reaches its 2.4 GHz p-state after ~3us of
    back-to-back work; any gap drops it to 1.2 GHz. So the whole kernel is
    emitted as one continuous PE stream: K-projection chunks are interleaved
    INTO the V-projection and attention groups, score matmuls run two
    head-pair groups ahead of the attn@v matmuls, and the exp/mult chain
    runs on ACT/DVE far off the PE critical path.
  * exp(attn_bias) * (mask ? 0 : 1) is precomputed on the HOST in bf16.
    The device computes e = exp(scores) [ACT, N=1024 across a 2-bank PSUM
    pair] then e *= eb in place [DVE, bf16x2 mode] - no fp32 tensor_tensor
    ADD (1x mode) and no mask handling on the device.
  * All PSUM work uses [128, 1024] two-bank pair tiles so activations
    amortize ACT's 352-cycle fixed overhead. PSUM budget: flowing pairs
    (2x2 banks) + held K-projection pair (2) + attn@v accumulators (2) = 8.
  * Softmax denominators come from the v65 ones-column; reciprocals run on
    DVE over a [64, 64] DRAM-reshaped view (64 elems/lane, not 512/lane).
"""
import sys

sys.path.insert(0, "/opt/trn_rl_repo")

import numpy as np
import ml_dtypes

import concourse.bass as bass
import concourse.bacc as bacc
import concourse.tile as tile
from concourse import mybir
from concourse.bass_utils import run_bass_kernel_spmd


def _pbcast(ap, nparts):
    """View `ap` (a [1, N] row) replicated across nparts partitions via a
    0-stride partition dim — DMA-source only."""
    row = ap
    return bass.AP(tensor=row.tensor, offset=row.offset,
                   ap=[[0, nparts]] + [list(d) for d in row.ap[1:]])


def _dview(ap, p, f):
    """Flat [p, f] view of a contiguous DRAM region starting at `ap`."""
    return bass.AP(tensor=ap.tensor, offset=ap.offset, ap=[[f, p], [1, f]])

F32 = mybir.dt.float32
BF16 = mybir.dt.bfloat16
Act = mybir.ActivationFunctionType
Alu = mybir.AluOpType
NPBF16 = ml_dtypes.bfloat16

B, T, S, E, H, D = 4, 1024, 1024, 1024, 16, 64
HP = H // 2          # head pairs
TS = T // 2          # per-core query rows (t-shard)
ET = E // 128        # 128-row tiles of the embed dim
SCALING = D ** -0.5

_CACHE = {}


def build_nc():
    nc = bacc.Bacc("TRN2", target_bir_lowering=False, debug=False, num_devices=8)

    qin_d = nc.dram_tensor("qin", [E, TS], BF16, kind="ExternalInput").ap()
    kin_d = nc.dram_tensor("kin", [E, S], BF16, kind="ExternalInput").ap()
    vin_d = nc.dram_tensor("vin", [E, S], BF16, kind="ExternalInput").ap()
    # exp(bias)*mask, [hp*8+j, s-in-tile, hh*512 + t]
    eb_d = nc.dram_tensor("ebT", [HP * 8, 128, 2 * TS], BF16,
                          kind="ExternalInput").ap()
    wq_d = nc.dram_tensor("wqt", [E, E], BF16, kind="ExternalInput").ap()
    wk_d = nc.dram_tensor("wkt", [E, E], BF16, kind="ExternalInput").ap()
    wv_d = nc.dram_tensor("wvt", [E, E], BF16, kind="ExternalInput").ap()
    wo_d = nc.dram_tensor("wot", [E, E], BF16, kind="ExternalInput").ap()
    bq_d = nc.dram_tensor("bqs", [128, 8], F32, kind="ExternalInput").ap()
    bk_d = nc.dram_tensor("bks", [128, 8], F32, kind="ExternalInput").ap()
    bv_d = nc.dram_tensor("bvr", [1, E], BF16, kind="ExternalInput").ap()
    bo_d = nc.dram_tensor("bor", [1, E], BF16, kind="ExternalInput").ap()
    out_d = nc.dram_tensor("out", [TS, E], F32, kind="ExternalOutput").ap()

    with tile.TileContext(nc) as tc:
        with tc.tile_pool(name="consts", bufs=1) as consts, \
             tc.tile_pool(name="wring", bufs=8) as wring, \
             tc.tile_pool(name="wkp", bufs=1) as wkp, \
             tc.tile_pool(name="kinp", bufs=1) as kinp, \
             tc.tile_pool(name="qvin", bufs=16) as qvin, \
             tc.tile_pool(name="persist", bufs=1) as persist, \
             tc.tile_pool(name="ebp", bufs=8) as ebp, \
             tc.tile_pool(name="emp", bufs=18) as emp, \
             tc.tile_pool(name="small", bufs=2) as small, \
             tc.tile_pool(name="osbp", bufs=2) as osbp, \
             tc.tile_pool(name="dramp", bufs=1, space="DRAM") as dramp, \
             tc.tile_pool(name="pssc", bufs=2, space="PSUM") as pssc, \
             tc.tile_pool(name="psot", bufs=1, space="PSUM") as psot:

            # ---- constants (bvb/bob DMAs deferred; see stages B/C) ----
            bq_sb = consts.tile([128, 8], F32, tag="bq")
            bk_sb = consts.tile([128, 8], F32, tag="bk")
            bvb = consts.tile([128, E], BF16, tag="bvb")  # bv bcast over parts
            bob = consts.tile([128, E], BF16, tag="bob")  # bo bcast over parts

            # ---- persistent products ----
            qT = [persist.tile([128, TS], BF16, tag=f"qt{hp}", name=f"qt{hp}")
                  for hp in range(HP)]
            kT = [[persist.tile([128, 512], BF16, tag=f"kt{hp}_{sh}",
                                name=f"kt{hp}_{sh}")
                   for sh in range(2)] for hp in range(HP)]
            v65 = [persist.tile([128, H, 65], BF16, tag=f"v65_{j}",
                                name=f"v65_{j}")
                   for j in range(8)]
            # unnormalized oT (f32) and normalized bf16 copy for out-proj
            otn = [persist.tile([128, TS], F32, tag=f"otn{et}", name=f"otn{et}")
                   for et in range(ET)]
            otn2 = [persist.tile([128, TS], BF16, tag=f"otn2_{et}",
                                 name=f"otn2_{et}")
                    for et in range(ET)]
            den_dram = dramp.tile([16, TS], F32, tag="dend", name="dend")
            rcp_dram = dramp.tile([16, TS], F32, tag="rcpd", name="rcpd")

            # ---- weight / input loads (stage A prefetch) ----
            # First-needed tiles are split into column chunks across several
            # DMA queues so the first matmul can start ~2us after queue
            # start-up instead of waiting one queue to stream 256KB.
            def chunked_load(t_, src, n):
                w = src.ap[-1][1]
                for ch in range(n):
                    lo, hi = w * ch // n, w * (ch + 1) // n
                    nc.sync.dma_start(out=t_[:, lo:hi], in_=src[:, lo:hi])

            wq_sb = [wring.tile([128, E], BF16, tag="w", name="w")
                     for _ in range(ET)]
            qin_sb = [qvin.tile([128, TS], BF16, tag="qv", name="qv")
                      for _ in range(ET)]
            wk_sb = [wkp.tile([128, E], BF16, tag=f"wk{et}", name=f"wk{et}")
                     for et in range(ET)]
            kin_sb = [[kinp.tile([128, 512], BF16, tag=f"ki{sh}_{et}",
                                 name=f"ki{sh}_{et}") for et in range(ET)]
                      for sh in range(2)]
            chunked_load(qin_sb[0], qin_d[0:128, :], 4)
            chunked_load(wq_sb[0], wq_d[0:128, :], 4)
            chunked_load(qin_sb[1], qin_d[128:256, :], 2)
            chunked_load(wq_sb[1], wq_d[128:256, :], 2)
            nc.sync.dma_start(out=bq_sb, in_=bq_d)
            nc.sync.dma_start(out=bk_sb, in_=bk_d)
            for et in range(2, ET):
                nc.sync.dma_start(out=qin_sb[et],
                                  in_=qin_d[et * 128:(et + 1) * 128, :])
                nc.sync.dma_start(out=wq_sb[et],
                                  in_=wq_d[et * 128:(et + 1) * 128, :])
            for et in range(ET):
                nc.sync.dma_start(out=wk_sb[et],
                                  in_=wk_d[et * 128:(et + 1) * 128, :])
                nc.sync.dma_start(out=kin_sb[0][et],
                                  in_=kin_d[et * 128:(et + 1) * 128, 0:512])
            for et in range(ET):
                nc.sync.dma_start(out=kin_sb[1][et],
                                  in_=kin_d[et * 128:(et + 1) * 128, 512:1024])

            # v65 ones columns (softmax denominator trick)
            for j in range(8):
                nc.vector.memset(v65[j][:, :, 64:65], 1.0)

            def k_proj_mms(hp, ps, sh, et0, n=2):
                """n et-steps of K[hp]'s sh-half into pair half sh."""
                for et in range(et0, et0 + n):
                    nc.tensor.matmul(ps[:, sh * 512:(sh + 1) * 512],
                                     wk_sb[et][:, hp * 128:(hp + 1) * 128],
                                     kin_sb[sh][et], start=(et == 0),
                                     stop=(et == ET - 1))

            def k_evac(hp, ps, engine):
                for sh in range(2):
                    if engine == "act":
                        nc.scalar.activation(kT[hp][sh],
                                             ps[:, sh * 512:(sh + 1) * 512],
                                             Act.Identity,
                                             bias=bk_sb[:, hp:hp + 1])
                    else:
                        nc.vector.tensor_scalar(
                            out=kT[hp][sh], in0=ps[:, sh * 512:(sh + 1) * 512],
                            scalar1=bk_sb[:, hp:hp + 1], scalar2=None,
                            op0=Alu.add)

            # ---- stage A: Q projection with K[0..1] woven in ----
            def q_pair(hpp):
                ps = pssc.tile([128, 1024], F32, tag="pair", name="pair")
                for h2 in range(2):
                    hp = 2 * hpp + h2
                    for et in range(ET):
                        nc.tensor.matmul(ps[:, h2 * 512:(h2 + 1) * 512],
                                         wq_sb[et][:, hp * 128:(hp + 1) * 128],
                                         qin_sb[et], start=(et == 0),
                                         stop=(et == ET - 1))
                for h2 in range(2):
                    hp = 2 * hpp + h2
                    nc.scalar.activation(qT[hp], ps[:, h2 * 512:(h2 + 1) * 512],
                                         Act.Identity, bias=bq_sb[:, hp:hp + 1])

            q_pair(0)
            kps = psot.tile([128, 1024], F32, tag="kpair", name="kpair")
            for sh in range(2):
                k_proj_mms(0, kps, sh, 0, n=ET)
            k_evac(0, kps, "act")
            q_pair(1)
            q_pair(2)
            kps = psot.tile([128, 1024], F32, tag="kpair", name="kpair")
            for sh in range(2):
                k_proj_mms(1, kps, sh, 0, n=ET)
            k_evac(1, kps, "act")
            q_pair(3)

            # ---- attention building blocks ----
            e2m_tiles = {}

            def s_pair(hp, j):
                """Scores for both heads of hp at s-tile j -> exp -> *=eb."""
                sh, sl = j // 4, j % 4
                eb = ebp.tile([128, 1024], BF16, tag="eb", name="eb")
                nc.sync.dma_start(out=eb, in_=eb_d[hp * 8 + j])
                ps = pssc.tile([128, 1024], F32, tag="pair", name="pair")
                for hh in range(2):
                    nc.tensor.matmul(
                        ps[:, hh * 512:(hh + 1) * 512],
                        kT[hp][sh][hh * 64:(hh + 1) * 64,
                                   sl * 128:(sl + 1) * 128],
                        qT[hp][hh * 64:(hh + 1) * 64, :],
                        start=True, stop=True,
                        tile_position=(hh * 64, 0))
                em = emp.tile([128, 1024], BF16, tag="em", name="em")
                nc.scalar.activation(em, ps, Act.Exp)
                nc.vector.tensor_tensor(out=em, in0=em, in1=eb, op=Alu.mult)
                e2m_tiles[(hp, j)] = em

            def o_mm(hp, j, poT):
                em = e2m_tiles.pop((hp, j))
                for hh in range(2):
                    h = 2 * hp + hh
                    nc.tensor.matmul(poT[hh], v65[j][:, h, :],
                                     em[:, hh * 512:(hh + 1) * 512],
                                     start=(j == 0), stop=(j == 7))

            def po_evac(hp, poT):
                for hh in range(2):
                    h = 2 * hp + hh
                    nc.vector.tensor_copy(out=otn[hp][hh * 64:(hh + 1) * 64, :],
                                          in_=poT[hh][0:64, :])
                    dst = small.tile([65, TS], F32, tag="dst", name="dst")
                    nc.vector.tensor_copy(out=dst[64:65, :],
                                          in_=poT[hh][64:65, :])
                    nc.sync.dma_start(out=den_dram[h:h + 1, :],
                                        in_=dst[64:65, :])

            def norm_mini(hp):
                """Reciprocal + broadcast + normalize for head-pair hp,
                emitted right after po_evac(hp) so otn2[hp] is ready ~3us
                after the group ends. All DMAs ride the gpsimd (software
                DGE) path so they never contend with the eb/weight stream."""
                deng = small.tile([16, 64], F32, tag="deng", name="deng")
                nc.sync.dma_start(out=deng,
                                    in_=_dview(den_dram[2 * hp:2 * hp + 2, :],
                                               16, 64))
                rcpg = small.tile([16, 64], F32, tag="rcpg", name="rcpg")
                nc.vector.reciprocal(out=rcpg, in_=deng)
                nc.sync.dma_start(out=_dview(rcp_dram[2 * hp:2 * hp + 2, :],
                                               16, 64),
                                    in_=rcpg)
                bc = small.tile([128, TS], F32, tag="bc", name="bc")
                row = rcp_dram[2 * hp:2 * hp + 2, :]
                nc.sync.dma_start(
                    out=bc,
                    in_=bass.AP(tensor=row.tensor, offset=row.offset,
                                ap=[[TS, 2], [0, 64], [1, TS]]))
                nc.vector.tensor_tensor(out=otn2[hp], in0=otn[hp], in1=bc,
                                        op=Alu.mult)

            # ---- stage B: V projection + K[2..3] + scores for hp 0..1 ----
            chunked_load(bvb, _pbcast(bv_d, 128), 2)
            wv_sb = []
            for et in range(ET):
                w = wring.tile([128, E], BF16, tag="w", name="w")
                nc.sync.dma_start(out=w, in_=wv_d[et * 128:(et + 1) * 128, :])
                wv_sb.append(w)
            vin_sb = [[None] * ET for _ in range(2)]
            for sh in range(2):
                for et in range(ET):
                    t_ = qvin.tile([128, 512], BF16, tag="qv", name="qv")
                    nc.sync.dma_start(
                        out=t_, in_=vin_d[et * 128:(et + 1) * 128,
                                          sh * 512:(sh + 1) * 512])
                    vin_sb[sh][et] = t_

            spairsB = [(0, j) for j in range(8)] + [(1, j) for j in range(8)]
            kps = psot.tile([128, 1024], F32, tag="kpair", name="kpair")
            vpair = None
            for idx in range(16):
                j, half = idx // 2, idx % 2
                sh, st = j // 4, j % 4
                if half == 0:
                    vpair = pssc.tile([128, 1024], F32, tag="pair", name="pair")
                for et in range(ET // 2 * half, ET // 2 * (half + 1)):
                    for ih in range(2):
                        nc.tensor.matmul(
                            vpair[:, ih * 512:(ih + 1) * 512],
                            vin_sb[sh][et][:, st * 128:(st + 1) * 128],
                            wv_sb[et][:, ih * 512:(ih + 1) * 512],
                            start=(et == 0), stop=(et == ET - 1))
                if half == 1:
                    nc.vector.tensor_tensor(
                        out=v65[j][:, :, 0:64],
                        in0=vpair.rearrange("p (h d) -> p h d", h=16),
                        in1=bvb.rearrange("p (h d) -> p h d", h=16),
                        op=Alu.add)
                # K[2] over idx 0..7, K[3] over idx 8..15 (2 mms per idx)
                if idx == 8:
                    k_evac(2, kps, "dve")
                    kps = psot.tile([128, 1024], F32, tag="kpair",
                                    name="kpair")
                khp = 2 + idx // 8
                k_proj_mms(khp, kps, (idx % 8) // 4, 2 * (idx % 4), n=2)
                s_pair(*spairsB[idx])
            k_evac(3, kps, "dve")

            # ---- stage C: attention groups; K[4..7] and out-proj wave A
            # (et 0..3 of tt=g-4, using PSUM freed by the K projections)
            # interleave into the groups ----
            chunked_load(bob, _pbcast(bo_d, 128), 2)
            wo_sb = []
            for et in range(ET):
                w = wring.tile([128, E], BF16, tag="w", name="w")
                nc.sync.dma_start(out=w, in_=wo_d[et * 128:(et + 1) * 128, :])
                wo_sb.append(w)

            def out_mm1(tt, ps, et, start_et, stop_et):
                for ih in range(2):
                    nc.tensor.matmul(
                        ps[:, ih * 512:(ih + 1) * 512],
                        otn2[et][:, tt * 128:(tt + 1) * 128],
                        wo_sb[et][:, ih * 512:(ih + 1) * 512],
                        start=(et == start_et), stop=(et == stop_et))

            def out_store(tt, ps, part=None):
                for ih in range(2):
                    o = osbp.tile([128, 512], F32, tag="osb", name="osb")
                    nc.vector.tensor_tensor(
                        out=o, in0=ps[:, ih * 512:(ih + 1) * 512],
                        in1=(bob if part is None else part)[
                            :, ih * 512:(ih + 1) * 512], op=Alu.add)
                    nc.sync.dma_start(
                        out=out_d[tt * 128:(tt + 1) * 128,
                                  ih * 512:(ih + 1) * 512], in_=o)

            wave_ps = {}
            parts = {}
            for g in range(8):
                poT = [psot.tile([65, 512], F32, tag=f"ot{hh}", name=f"ot{hh}")
                       for hh in range(2)]
                kpair = None
                if g + 4 <= 7:
                    kpair = psot.tile([128, 1024], F32, tag="kpair",
                                      name="kpair")
                for j in range(8):
                    # last slot: o first so po_evac leads the group's DVE tail
                    if j == 7:
                        if kpair is not None:
                            k_proj_mms(g + 4, kpair, j // 4, 2 * (j % 4))
                        o_mm(g, j, poT)
                        po_evac(g, poT)
                        norm_mini(g)
                        if g + 2 <= 7:
                            s_pair(g + 2, j)
                    else:
                        if g + 2 <= 7:
                            s_pair(g + 2, j)
                        if kpair is not None:
                            k_proj_mms(g + 4, kpair, j // 4, 2 * (j % 4))
                        o_mm(g, j, poT)
                    # out-proj wave A: tt=g-4 covers et 0..g-1 (otn2[et] is
                    # ready ~one group after et's own group), one et per slot.
                    # tt0/tt1 use the kpair banks (K projections done), tt2/
                    # tt3 the pair ring (score pairs done after group 5).
                    if g >= 4 and j < g:
                        tt = g - 4
                        if j == 0:
                            wave_ps[tt] = (
                                psot.tile([128, 1024], F32, tag="kpair",
                                          name="kpair") if tt in (0, 1)
                                else pssc.tile([128, 1024], F32, tag="pair",
                                               name="pair"))
                        # tt0/tt2 close their group (partial spilled);
                        # tt1/tt3 stay open and finish in stage D.
                        out_mm1(tt, wave_ps[tt], j, 0,
                                g - 1 if tt in (0, 2) else ET - 1)
                if kpair is not None:
                    k_evac(g + 4, kpair, "dve")
                # tt0 would block tt1's wave alloc on the kpair ring (and tt2
                # tt3's on the pair ring): spill tt0/tt2 partials to SBUF;
                # tt1/tt3 pairs stay held into stage D.
                if g in (4, 6):
                    tt = g - 4
                    part = osbp.tile([128, 1024], BF16, tag="part",
                                     name="part")
                    nc.vector.tensor_tensor(out=part, in0=wave_ps[tt],
                                            in1=bob, op=Alu.add)
                    parts[tt] = part

            # ---- stage D: out-proj tails (tt0: et4.., tt1: et5.., tt3: et7,
            # then tt2: et6.. on tt3's freed pair-ring slot) ----
            tail_ps = {1: wave_ps[1], 3: wave_ps[3]}
            tail_ps[0] = pssc.tile([128, 1024], F32, tag="pair", name="pair")
            for et in range(4, 7):
                out_mm1(0, tail_ps[0], et, 4, ET - 1)
            for et in range(5, 7):
                out_mm1(1, tail_ps[1], et, 0, ET - 1)
            out_mm1(0, tail_ps[0], 7, 4, ET - 1)
            out_store(0, tail_ps[0], part=parts[0])
            out_mm1(1, tail_ps[1], 7, 0, ET - 1)
            out_store(1, tail_ps[1])
            out_mm1(3, tail_ps[3], 7, 0, ET - 1)
            out_store(3, tail_ps[3])
            tail_ps[2] = pssc.tile([128, 1024], F32, tag="pair", name="pair")
            for et in range(6, ET):
                out_mm1(2, tail_ps[2], et, 6, ET - 1)
            out_store(2, tail_ps[2], part=parts[2])

    nc.compile()
    return nc


def _prepare_in_maps(query, key, value, key_padding_mask, attn_bias,
                     wq, bq, wk, bk, wv, bv, wo, bo):
    wqt = (np.ascontiguousarray(wq.T) * SCALING).astype(NPBF16)
    wkt = np.ascontiguousarray(wk.T).astype(NPBF16)
    wvt = np.ascontiguousarray(wv.T).astype(NPBF16)
    wot = np.ascontiguousarray(wo.T).astype(NPBF16)
    bqs = np.ascontiguousarray((bq * SCALING).reshape(8, 128).T)
    bks = np.ascontiguousarray(bk.astype(np.float32).reshape(8, 128).T)
    bvr = np.ascontiguousarray(bv.astype(NPBF16))[None, :]
    bor = np.ascontiguousarray(bo.astype(NPBF16))[None, :]

    kin_b = [np.ascontiguousarray(key[b_].T).astype(NPBF16) for b_ in range(B)]
    vin_b = [np.ascontiguousarray(value[b_].T).astype(NPBF16) for b_ in range(B)]

    # exp(bias) * (mask ? 0 : 1), bf16, per-core layout
    ebf = np.exp(attn_bias.astype(np.float32)).reshape(B, H, T, S)
    ebf = ebf * (~key_padding_mask)[:, None, None, :].astype(np.float32)

    in_maps = []
    for c in range(8):
        b_, th = c // 2, c % 2
        qin = np.ascontiguousarray(
            query[b_, th * TS:(th + 1) * TS, :].T).astype(NPBF16)
        sl = ebf[b_, :, th * TS:(th + 1) * TS, :]      # [16, 512t, 1024s]
        x = sl.reshape(8, 2, TS, 8, 128)               # [hp, hh, t, j, s128]
        x = x.transpose(0, 3, 4, 1, 2)                 # [hp, j, s128, hh, t]
        ebT = np.ascontiguousarray(x).reshape(64, 128, 1024).astype(NPBF16)
        in_maps.append({
            "qin": qin, "kin": kin_b[b_], "vin": vin_b[b_],
            "ebT": ebT,
            "wqt": wqt, "wkt": wkt, "wvt": wvt, "wot": wot,
            "bqs": bqs, "bks": bks, "bvr": bvr, "bor": bor,
        })
    return in_maps


def kernel(query, key, value, key_padding_mask, attn_bias,
           wq, bq, wk, bk, wv, bv, wo, bo, _run_kwargs=None):
    query = np.asarray(query, dtype=np.float32)
    key = np.asarray(key, dtype=np.float32)
    value = np.asarray(value, dtype=np.float32)
    key_padding_mask = np.asarray(key_padding_mask)
    attn_bias = np.asarray(attn_bias, dtype=np.float32)
    wq, bq = np.asarray(wq, np.float32), np.asarray(bq, np.float32)
    wk, bk = np.asarray(wk, np.float32), np.asarray(bk, np.float32)
    wv, bv = np.asarray(wv, np.float32), np.asarray(bv, np.float32)
    wo, bo = np.asarray(wo, np.float32), np.asarray(bo, np.float32)

    if "nc" not in _CACHE:
        _CACHE["nc"] = build_nc()
    nc = _CACHE["nc"]

    in_maps = _prepare_in_maps(query, key, value, key_padding_mask, attn_bias,
                               wq, bq, wk, bk, wv, bv, wo, bo)
    res = run_bass_kernel_spmd(nc, in_maps, core_ids=list(range(8)),
                               **(_run_kwargs or {}))
    _CACHE["last_results"] = res

    out = np.empty((B, T, E), dtype=np.float32)
    for c in range(8):
        b_, th = c // 2, c % 2
        out[b_, th * TS:(th + 1) * TS, :] = res.results[c]["out"]
    return out
